# revision 1
# baseline (speedup 1.0000x reference)
"""Expert-parallel grouped GEMM (MoE) kernel for Trainium2.

Problem: inputs [65536, 1024] sorted by expert (8192 tokens/expert),
weight [8, 512, 1024]; out[t] = x[t] @ W[expert(t)].T -> [65536, 512].

Sharding: expert-parallel across 8 NeuronCores. Tokens are already sorted
by expert and expert_size is static, so core e simply takes token rows
[e*8192:(e+1)*8192] and weight[e] - no all-to-all needed.

Device kernel (per core): one [8192,1024] @ [1024,512] GEMM.
- Both matmul operands need the contraction dim (I) on the SBUF partition
  axis, so the host pre-transposes x -> xT [I, S] and W[e] -> wT [I, O].
- fp16 operands: the PE runs 2-byte dtypes at 1 cycle/row (fp32/fp32r
  take 2-4x), and fp16's 10-bit mantissa keeps the end-to-end error at
  ~4e-4 for N(0,1) data (bf16 would be ~2e-3). Accumulation is fp32 in
  PSUM.
- Weight-stationary loop: the PE's stationary operand is a 128x128 tile
  of W, and consecutive matmuls stream different token chunks through
  it. bacc emits one LDWEIGHTS per matmul with no dedup (a measured
  ~107ns serial cost per matmul, ~40% overhead at N=512); a post-compile
  pass strips LDWEIGHTS whose weight AP matches the previous one on the
  PE stream, carrying their semaphore waits/updates onto the next PE
  instruction. This is the documented-valid "standalone LDWEIGHTS +
  non-self-loading matmul" pattern for 2-byte dtypes.
- The output is produced transposed (psum [o_tile, s_chunk]); the device
  writes outT [O, S] fp16 and the host transposes/upcasts. This is what
  makes the weight reuse possible, and the out DMA lines get long (4 KB).
- wT stays resident in SBUF; xT streams in prefetched blocks on the SP
  HWDGE ring; outputs leave on the ACT HWDGE ring.
"""

import numpy as np

E = 8          # experts == cores
O = 512        # out_features
I = 1024       # in_features
S = 8192       # tokens per expert
KT = I // 128  # k-tiles (contraction)
OT = O // 128  # o-tiles
SC = 512       # tokens per matmul (moving free dim)
S_BLK = 2048   # max tokens per streamed x block
BLOCKS = (512, 1536, 2048, 2048, 1536, 512)  # ramp up AND down, sums to S
X_BUFS = 4     # x block buffers (prefetch depth)
IN_DT = "float16"   # matmul operand dtype: float16 | float32r
DEDUP = False  # strip redundant LDWEIGHTS post-compile (no win measured)
STRUCT = "xstat"    # wstat: W stationary, out transposed | xstat: x stationary
OUT_B = 4      # xstat only: t-tiles batched per output DMA

_cache = {}


def _merge_sync(mybir, inst, waits, updates):
    si = inst.sync_info
    if si is None:
        inst.sync_info = mybir.SyncInfo(on_wait=list(waits), on_update=list(updates))
    else:
        si.on_wait = list(waits) + list(si.on_wait)
        si.on_update = list(si.on_update) + list(updates)


def _dedup_ldweights(nc):
    """Remove InstLdweights that reload the identical weight tile.

    Tracks the last-loaded weight signature along each block's PE stream;
    resets at any PE instruction other than a plain matmul (branches,
    drains, barriers, transposes), so loop back-edges stay conservative.
    Waits/updates of removed loads move to the next kept PE instruction.
    """
    from concourse import mybir

    removed = 0
    for fn in nc.m.functions:
        for blk in fn.blocks:
            insts = blk.instructions
            keep = []
            last_sig = None
            pend_w, pend_u = [], []
            for inst in insts:
                if inst.engine != mybir.EngineType.PE:
                    keep.append(inst)
                    continue
                if isinstance(inst, mybir.InstLdweights) and not inst.is_transpose:
                    a = inst.ins[0]
                    sig = (a.memref, a.offset, str(a.ap),
                           str(inst.tile_position), str(inst.perf_mode))
                    if sig == last_sig:
                        si = inst.sync_info
                        if si is not None:
                            pend_w.extend(list(si.on_wait))
                            pend_u.extend(list(si.on_update))
                        removed += 1
                        continue
                    last_sig = sig
                elif not (isinstance(inst, mybir.InstMatmult)
                          and not inst.is_transpose):
                    last_sig = None
                if pend_w or pend_u:
                    _merge_sync(mybir, inst, pend_w, pend_u)
                    pend_w, pend_u = [], []
                keep.append(inst)
            assert not pend_w and not pend_u, "dangling sync from removed ldweights"
            insts[:] = keep
    return removed


def _build_nc(repeats=1, loop=0, idle=0):
    import concourse.bass as bass
    import concourse.tile as tile
    from concourse import bacc, mybir
    from contextlib import nullcontext

    in_dt = getattr(mybir.dt, IN_DT)
    blocks = []  # (start_token, n_tokens)
    pos = 0
    for sz in BLOCKS:
        blocks.append((pos, sz))
        pos += sz
    assert pos == S and all(sz % SC == 0 and sz <= S_BLK for _, sz in blocks)

    nc = bacc.Bacc("TRN2", target_bir_lowering=False, debug=False)
    xT = nc.dram_tensor("xT", [I, S], in_dt, kind="ExternalInput")
    wT = nc.dram_tensor("wT", [I, O], in_dt, kind="ExternalInput")
    out_shape = [O, S] if STRUCT == "wstat" else [S, O]
    outT = nc.dram_tensor("out", out_shape, mybir.dt.float16, kind="ExternalOutput")
    if idle:
        ping = nc.dram_tensor("ping", [1, 8], mybir.dt.float16)
        pong = nc.dram_tensor("pong", [1, 8], mybir.dt.float16)

    with tile.TileContext(nc) as tc:
        with (
            tc.tile_pool(name="wpool", bufs=1) as wpool,
            tc.tile_pool(name="xpool", bufs=X_BUFS) as xpool,
            tc.tile_pool(name="opool", bufs=4) as opool,
            tc.tile_pool(name="psum", bufs=8, space=bass.MemorySpace.PSUM) as psum_pool,
        ):
            wt = wpool.tile([128, KT * O], in_dt)

            def load_block(blk, with_weights=False):
                # with_weights: interleave the resident-weight k-tile loads
                # with this block's stripes so the first matmul (needs only
                # wt[k=0] + stripe[k=0]) starts ~5us earlier than with a
                # serial full-weight prefix.
                s0, sz = blk
                xblk = xpool.tile([128, KT * sz], in_dt, tag="xblk")
                for k in range(KT):
                    if with_weights:
                        nc.sync.dma_start(wt[:, k * O:(k + 1) * O],
                                          wT[k * 128:(k + 1) * 128, :])
                    nc.sync.dma_start(
                        xblk[:, k * sz:(k + 1) * sz],
                        xT[k * 128:(k + 1) * 128, s0:s0 + sz],
                    )
                return xblk

            last_ot = [None]

            def compute_block_wstat(blk, xblk):
                s0, sz = blk
                n_sc = sz // SC
                for o in range(OT):
                    pss = [psum_pool.tile([128, SC], mybir.dt.float32,
                                          name="ps", tag="ps")
                           for _ in range(n_sc)]
                    for k in range(KT):
                        lw = wt[:, k * O + o * 128: k * O + (o + 1) * 128]
                        for sc in range(n_sc):
                            nc.tensor.matmul(
                                pss[sc][:],
                                lw,
                                xblk[:, k * sz + sc * SC: k * sz + (sc + 1) * SC],
                                start=(k == 0),
                                stop=(k == KT - 1),
                            )
                    ot = opool.tile([128, sz], mybir.dt.float16, tag="ot")
                    for sc in range(n_sc):
                        nc.vector.tensor_copy(ot[:, sc * SC:(sc + 1) * SC], pss[sc][:])
                    nc.scalar.dma_start(
                        outT[o * 128:(o + 1) * 128, s0:s0 + sz], ot[:])
                    last_ot[0] = ot

            def compute_block_xstat(blk, xblk):
                s0, sz = blk
                for tg in range(sz // 128 // OUT_B):
                    ot = opool.tile([128, OUT_B, O], mybir.dt.float16, tag="ot")
                    for ti in range(OUT_B):
                        t = tg * OUT_B + ti
                        ps = psum_pool.tile([128, O], mybir.dt.float32,
                                            name="ps", tag="ps")
                        for k in range(KT):
                            nc.tensor.matmul(
                                ps[:],
                                xblk[:, k * sz + t * 128: k * sz + (t + 1) * 128],
                                wt[:, k * O:(k + 1) * O],
                                start=(k == 0),
                                stop=(k == KT - 1),
                            )
                        nc.vector.tensor_copy(ot[:, ti, :], ps[:])
                    g0 = s0 + tg * OUT_B * 128
                    dst = outT[g0:g0 + OUT_B * 128, :].rearrange(
                        "(t p) o -> p t o", p=128)
                    nc.scalar.dma_start(dst, ot[:])
                    last_ot[0] = ot[:, 0, :]

            compute_block = (compute_block_wstat if STRUCT == "wstat"
                             else compute_block_xstat)

            loop_cm = (
                tc.For_i(0, loop, 1,
                         hint_engines=(mybir.EngineType.PE, mybir.EngineType.SP,
                                       mybir.EngineType.DVE))
                if loop else nullcontext()
            )
            with loop_cm:
                for _ in range(repeats):
                    pending = []  # (blk, xblk) loaded but not yet computed
                    for bi, blk in enumerate(blocks):
                        pending.append((blk, load_block(blk, with_weights=bi == 0)))
                        if len(pending) >= X_BUFS:
                            compute_block(*pending.pop(0))
                    for blk, xblk in pending:
                        compute_block(blk, xblk)
                # low-power idle: dependent tiny DMA ping-pong through one
                # SBUF tile (Tile tracks the tile's RAW/WAR deps, so the
                # copies serialize on each other's completion latency).
                # The first copy reads the gemm's final output tile, so the
                # idle runs strictly AFTER the gemm instead of alongside it,
                # and the per-iteration span is gemm_span + idle_span.
                # Keeps average chip power low so duty-cycled benchmarks see
                # the unthrottled PE clock.
                if idle:
                    idle_t = wpool.tile([1, 8], mybir.dt.float16, name="idle_t")
                    if last_ot[0] is not None:
                        nc.sync.dma_start(idle_t[:], last_ot[0][0:1, 0:8])
                    for i in range(idle):
                        if i % 2 == 0:
                            nc.sync.dma_start(pong[:], idle_t[:])
                        else:
                            nc.sync.dma_start(idle_t[:], ping[:])
    nc.compile()
    if DEDUP and repeats > 0:
        _dedup_ldweights(nc)
    return nc


def _get_nc(repeats=1, loop=0, idle=0):
    key = (repeats, loop, idle, BLOCKS, X_BUFS, SC, IN_DT, DEDUP, STRUCT, OUT_B)
    if key not in _cache:
        _cache[key] = _build_nc(repeats, loop, idle)
    return _cache[key]


def _np_in_dt():
    return np.float16 if IN_DT == "float16" else np.float32


def run(inputs, weight, trace=False, repeats=1, loop=0):
    """Shard, run on 8 cores, gather. Returns (out, BassKernelResults)."""
    from concourse.bass_utils import run_bass_kernel_spmd

    nc = _get_nc(repeats, loop)
    dt = _np_in_dt()
    in_maps = []
    for e in range(E):
        x_e = np.ascontiguousarray(inputs[e * S:(e + 1) * S, :].T.astype(dt))
        w_e = np.ascontiguousarray(weight[e].T.astype(dt))
        in_maps.append({"xT": x_e, "wT": w_e})
    res = run_bass_kernel_spmd(nc, in_maps, list(range(E)), trace=trace)
    outs = [res.results[e]["out"] for e in range(E)]
    if STRUCT == "wstat":
        outs = [o.T for o in outs]
    out = np.concatenate([o.astype(np.float32) for o in outs], axis=0)
    return out, res


def kernel(inputs, weight, expert_size):
    inputs = np.asarray(inputs, dtype=np.float32)
    weight = np.asarray(weight, dtype=np.float32)
    assert inputs.shape == (E * S, I) and weight.shape == (E, O, I)
    assert int(expert_size) == S
    out, _ = run(inputs, weight, trace=False)
    return out

